# revision 39
# baseline (speedup 1.0000x reference)
"""DocRED relation-extraction head on 8 Trainium2 NeuronCores (~39us).

Data-parallel over the batch axis: core b owns batch b's hidden_states slab
and its entity/pair indices; the classifier weights are replicated.

Instead of materializing rel = concat(subj, obj) [P, 2H] and computing
[P,2H] @ [2H,H] @ [H,97] per pair (32x redundant: only 32 distinct entities
feed 1024 pairs), project the entities first and gather at the end:

    repT    = mention-sum of 128 hidden rows, via indirect-DMA gather +
              8 matmuls against a block-ones matrix (fuses sum + transpose)
    proj    = rep @ [W1 | W2]   W streamed as the MOVING operand in N=512
              chunks (16 x 256KB fp16 slabs, the dominant DMA stream, at the
              HBM-pair roofline); 4 PSUM bank accumulators
    projT   = proj flipped via 2 batched DVE 32x32 stream-transposes per
              half + 4 partition-restack SBUF->SBUF DMAs
    eL1'    = [projT1 | dense_b].T @ out_w  [33, 98]  (row 32 = const row)
    eL2     = projT2.T @ out_w              [32, 98]
    logits[p] = eL1'[head[p]] + const + eL2[tail[p]], via ONE K=65-stacked
              one-hot matmul per 128-pair tile: rows 0-31 head one-hot,
              32-63 tail one-hot, 64 all-ones (adds const+out_b once).

Precision: fp32 matmul on trn2 lowers to 2 ISA passes (hi/lo), so matmul
inputs use float32r (single-pass, ~1.3e-4 rel per matmul), and dense_w /
hidden_states travel as fp16 (their magnitudes are well inside fp16 range;
halves the dominant DMA bytes). End-to-end vs the fp32 reference: ~4e-4
scale-relative error. Set MM_DT = f32 / W_DT = f32 for exact fp32 (~77us).
"""

import numpy as np
from contextlib import ExitStack

import concourse.bass as bass
import concourse.bacc as bacc
import concourse.tile as tile
import concourse.mybir as mybir
from concourse.bass_utils import run_bass_kernel_spmd

B, L, H, E, M, P, C = 8, 2048, 1024, 32, 4, 1024, 97
N_CORES = 8
HC = H // 128   # h-dim chunks (contraction of dense)
JC = H // 128   # j-dim chunks (output of dense / contraction of out proj)
PT = P // 128   # pair tiles
SLOT = E + 1    # projT slot width: 32 cols projT + 1 col dense_b chunk

f32 = mybir.dt.float32
f32r = mybir.dt.float32r
i32 = mybir.dt.int32

MM_DT = f32r    # dtype of matmul input tiles (f32r: 1-pass PE, ~1e-4 rel err)
W_DT = mybir.dt.float16  # dense_w + repT dtype: halves the dominant DMA
                         # stream (4MB vs 8MB); W values are ~N(0, 1/2048) so
                         # fp16's 10-bit mantissa costs ~5e-4 rel

CP = C + 1                # class dim padded to 98: f32r needs an even
                          # moving dim; the pad column is zero end to end

# constant-blob column layout
ONES0 = 1                 # [128, 32] mention-sum block-ones
DB0 = ONES0 + E           # [128, 8] dense_b chunks
OW0 = DB0 + HC            # [128, 8*98] out_w chunks (zero pad col each)
IOTA0 = OW0 + JC * CP     # [32, 1] iota column
OB0 = IOTA0 + 1           # [1, 98] out_b on row 0 (zero padded)
BLOBW = OB0 + CP

_CACHE = {}


def _build():
    nc = bacc.Bacc("TRN2", target_bir_lowering=False, debug=False)

    hs = nc.dram_tensor("hs", [L, H], W_DT, kind="ExternalInput").ap()
    pos = nc.dram_tensor("pos", [E * M, 1], i32, kind="ExternalInput").ap()
    onesb = nc.dram_tensor("onesb", [E * M, E], W_DT, kind="ExternalInput").ap()
    blob = nc.dram_tensor("blob", [128, BLOBW], MM_DT, kind="ExternalInput").ap()
    headrep = nc.dram_tensor("headrep", [E, P], MM_DT, kind="ExternalInput").ap()
    tailrep = nc.dram_tensor("tailrep", [E, P], MM_DT, kind="ExternalInput").ap()
    dw = nc.dram_tensor("dw", [2 * H, H], W_DT, kind="ExternalInput").ap()
    # output laid out [128, PT*C]: pair-tile t in columns t*C..(t+1)*C; one
    # contiguous DMA out, host reshapes to [P, C]
    out = nc.dram_tensor("out", [128, PT * C], f32, kind="ExternalOutput").ap()

    with tile.TileContext(nc) as tc, ExitStack() as ctx:
        sb = ctx.enter_context(tc.tile_pool(name="sb", bufs=1))
        wpool = ctx.enter_context(tc.tile_pool(name="w", bufs=12))
        opool = ctx.enter_context(tc.tile_pool(name="o", bufs=2))
        # One accumulation group per PSUM bank at a time (start=True clears
        # has_written for the whole bank) -> single 8-slot pool, bank per slot.
        pspool = ctx.enter_context(tc.tile_pool(name="ps", bufs=8, space="PSUM"))

        # ---- latency-critical inputs: pos + ones lead the SYNC ring so the
        # gather + stage A can run before the W stream floods the SDMA
        # engines; everything else rides the scalar ring.
        sb_pos = sb.tile([E * M, 1], i32)
        nc.sync.dma_start(sb_pos[:], pos[:])
        sb_ones = sb.tile([E * M, E], W_DT)
        nc.sync.dma_start(sb_ones[:], onesb[:])
        sb_blob = sb.tile([128, BLOBW], MM_DT)
        nc.scalar.dma_start(sb_blob[:], blob[:])
        sb_hr = sb.tile([E, P], MM_DT)
        nc.scalar.dma_start(sb_hr[:], headrep[:])
        sb_tr = sb.tile([E, P], MM_DT)
        nc.scalar.dma_start(sb_tr[:], tailrep[:])

        # ---- gather the 128 mention rows of hidden_states
        sb_g = sb.tile([E * M, H], W_DT)
        nc.gpsimd.indirect_dma_start(
            out=sb_g[:],
            out_offset=None,
            in_=hs[:],
            in_offset=bass.IndirectOffsetOnAxis(ap=sb_pos[:, :1], axis=0),
        )

        # ---- PE warm-up: the HAM clock gate holds an idle PE at 1.2 GHz and
        # needs ~3.4us of sustained activity to release to 2.4 GHz. Burn
        # discarded matmuls on a memset tile (no DMA dependency -> starts as
        # soon as the PE preamble ends) so stage B runs at full clock. f32
        # dummies lower to 2 ISA passes each: fewer instructions per us.
        wdum = sb.tile([128, E], f32)
        nc.vector.memset(wdum[:], 0.0)
        ps_warm = pspool.tile([E, E], f32, tag="ps")
        for i in range(90):
            nc.tensor.matmul(
                out=ps_warm[:],
                lhsT=wdum[:],
                rhs=wdum[:],
                start=True,
                stop=True,
            )

        # ---- stage A: entity_repT[h, e] = sum_m gathered[4e+m, h]
        # (mention-sum and transpose fused into 8 matmuls vs block-ones)
        sb_repT = sb.tile([128, HC * E], W_DT)
        for hc in range(HC):
            pa = pspool.tile([128, E], f32, tag="ps", name=f"pa{hc}")
            nc.tensor.matmul(
                out=pa[:],
                lhsT=sb_g[:, hc * 128:(hc + 1) * 128],
                rhs=sb_ones[:],
                start=True,
                stop=True,
            )
            nc.vector.tensor_copy(out=sb_repT[:, hc * E:(hc + 1) * E], in_=pa[:])

        # ---- stage B: proj = rep @ [W1 | W2] with W as the moving operand.
        sb_projT = sb.tile([128, 2 * JC * SLOT], MM_DT)
        ps_eL1 = pspool.tile([SLOT, CP], f32, tag="ps")
        ps_eL2 = pspool.tile([E, CP], f32, tag="ps")
        # dense_b chunks ride along as lhsT column 32 of the half-0 slots
        for jc in range(JC):
            nc.vector.tensor_copy(
                out=sb_projT[:, jc * SLOT + E:jc * SLOT + E + 1],
                in_=sb_blob[:, DB0 + jc:DB0 + jc + 1],
            )
        # both halves of the W stream back-to-back on PE (no C in between --
        # C waits on the DVE transposes and would bubble the PE otherwise)
        ps_blk = [pspool.tile([E, 512], f32, tag="ps", name=f"ps_blk{i}")
                  for i in range(4)]
        for half in range(2):
            for hc in range(HC):
                wt = wpool.tile([128, H], W_DT, tag="wslab",
                                name=f"wt{half}_{hc}")
                nc.sync.dma_start(
                    wt[:], dw[(half * HC + hc) * 128:(half * HC + hc + 1) * 128, :])
                for q in range(2):
                    nc.tensor.matmul(
                        out=ps_blk[half * 2 + q][:],
                        lhsT=sb_repT[:, hc * E:(hc + 1) * E],
                        rhs=wt[:, q * 512:(q + 1) * 512],
                        start=(hc == 0),
                        stop=(hc == HC - 1),
                    )
        # per half: 2 batched 32x32-block stream-transposes (16 blocks each),
        # then 4 small SBUF->SBUF DMAs restack the blocks into partitions
        # (the DMA also retypes f32 -> f32r; same 4-byte lanes)
        for half in range(2):
            stT = sb.tile([E, H], f32, name=f"stT{half}")
            for q in range(2):
                nc.vector.transpose(
                    out=stT[:, q * 512:(q + 1) * 512],
                    in_=ps_blk[half * 2 + q][:])
            hs0 = half * JC * SLOT
            st4 = stT[:].bitcast(MM_DT).rearrange("p (j b c) -> p j b c", b=4, c=32)
            for bl in range(4):
                # half 0 runs mid-W-stream: keep its restacks off the sync
                # ring (FIFO behind the W slabs). At half 1 both rings idle.
                eng = nc.sync if (half == 1 and bl % 2 == 1) else nc.scalar
                eng.dma_start(
                    out=sb_projT[bl * 32:(bl + 1) * 32, hs0:hs0 + JC * SLOT]
                    .rearrange("p (j s) -> p j s", s=SLOT)[:, :, :E],
                    in_=st4[:, :, bl, :],
                )
            eL, w_m = (ps_eL1, SLOT) if half == 0 else (ps_eL2, E)
            for jc in range(JC):
                slot = (half * JC + jc) * SLOT
                nc.tensor.matmul(
                    out=eL[:],
                    lhsT=sb_projT[:, slot:slot + w_m],
                    rhs=sb_blob[:, OW0 + jc * CP:OW0 + (jc + 1) * CP],
                    start=(jc == 0),
                    stop=(jc == JC - 1),
                )

        # ---- eL stack [65, 97] (partition offsets must be 32-aligned):
        # rows 0-31 = eL1, rows 32-63 = eL2, row 64 = dense_b @ out_w + out_b.
        sb_eL = sb.tile([2 * E + 1, CP], MM_DT)
        nc.vector.tensor_copy(out=sb_eL[:E, :], in_=ps_eL1[:E, :])
        nc.vector.tensor_copy(out=sb_eL[E:2 * E, :], in_=ps_eL2[:])
        nc.vector.tensor_add(
            out=sb_eL[2 * E:2 * E + 1, :], in0=ps_eL1[E:E + 1, :],
            in1=sb_blob[:1, OB0:OB0 + CP])

        # ---- stage D: stacked one-hot pair gather.
        # K rows 0-31: head one-hot; rows 32-63: tail; row 64: ones (const).
        sb_oh = sb.tile([2 * E + 1, P], MM_DT)
        nc.vector.tensor_tensor(
            out=sb_oh[:E, :],
            in0=sb_blob[:E, IOTA0:IOTA0 + 1].to_broadcast([E, P]),
            in1=sb_hr[:],
            op=mybir.AluOpType.is_equal,
        )
        nc.vector.tensor_tensor(
            out=sb_oh[E:2 * E, :],
            in0=sb_blob[:E, IOTA0:IOTA0 + 1].to_broadcast([E, P]),
            in1=sb_tr[:],
            op=mybir.AluOpType.is_equal,
        )
        # all-ones row via x==x (memset on an f32r tile fails the ISA check)
        nc.vector.tensor_tensor(
            out=sb_oh[2 * E:2 * E + 1, :],
            in0=sb_blob[:1, IOTA0:IOTA0 + 1].to_broadcast([1, P]),
            in1=sb_blob[:1, IOTA0:IOTA0 + 1].to_broadcast([1, P]),
            op=mybir.AluOpType.is_equal,
        )
        sb_out = opool.tile([128, PT * C], f32, bufs=1)
        for pt in range(PT):
            pl = pspool.tile([128, CP], f32, tag="ps", name=f"pl{pt}")
            nc.tensor.matmul(
                out=pl[:],
                lhsT=sb_oh[:, pt * 128:(pt + 1) * 128],
                rhs=sb_eL[:],
                start=True,
                stop=True,
            )
            nc.vector.tensor_copy(
                out=sb_out[:, pt * C:(pt + 1) * C], in_=pl[:, :C])
        # single contiguous store; host reshapes [128, 8*97] -> [1024, 97]
        nc.scalar.dma_start(out[:], sb_out[:])

    nc.compile()
    return nc


def get_compiled():
    if "nc" not in _CACHE:
        _CACHE["nc"] = _build()
    return _CACHE["nc"]


def make_in_maps(hidden_states, dense_w, dense_b, out_w, out_b,
                 entity_position_ids, head_tail_idxs):
    # inputs may arrive as jax arrays; normalize to host numpy first
    hidden_states = np.asarray(hidden_states)
    dense_w = np.asarray(dense_w)
    dense_b = np.asarray(dense_b)
    out_w = np.asarray(out_w)
    out_b = np.asarray(out_b)
    entity_position_ids = np.asarray(entity_position_ids)
    head_tail_idxs = np.asarray(head_tail_idxs)
    blob = np.zeros((128, BLOBW), np.float32)
    blob[:, DB0:DB0 + HC] = np.asarray(dense_b, np.float32).reshape(HC, 128).T
    owp = np.zeros((H, CP), np.float32)
    owp[:, :C] = np.asarray(out_w, np.float32)
    blob[:, OW0:OW0 + JC * CP] = (
        owp.reshape(JC, 128, CP).transpose(1, 0, 2).reshape(128, JC * CP))
    blob[:E, IOTA0] = np.arange(E, dtype=np.float32)
    blob[0, OB0:OB0 + C] = np.asarray(out_b, np.float32)  # col 97 stays 0
    dense_w = np.ascontiguousarray(dense_w, dtype=np.float16)
    in_maps = []
    for b in range(B):
        ht = head_tail_idxs[b].astype(np.float32)  # [P, 2]
        in_maps.append({
            "hs": np.ascontiguousarray(hidden_states[b], dtype=np.float16),
            "pos": np.ascontiguousarray(
                entity_position_ids[b].reshape(E * M, 1).astype(np.int32)),
            "onesb": np.repeat(np.eye(E, dtype=np.float16), M, axis=0),
            "blob": blob,
            "headrep": np.ascontiguousarray(
                np.broadcast_to(ht[None, :, 0], (E, P))),
            "tailrep": np.ascontiguousarray(
                np.broadcast_to(ht[None, :, 1], (E, P))),
            "dw": dense_w,
        })
    return in_maps


def kernel(hidden_states, dense_w, dense_b, out_w, out_b,
           entity_position_ids, head_tail_idxs, _trace=False, _trace_kwargs=None):
    nc = get_compiled()
    in_maps = make_in_maps(hidden_states, dense_w, dense_b, out_w, out_b,
                           entity_position_ids, head_tail_idxs)
    res = run_bass_kernel_spmd(
        nc, in_maps, core_ids=list(range(N_CORES)),
        trace=_trace, **(_trace_kwargs or {}),
    )
    outp = np.concatenate(
        [res.results[i]["out"].reshape(128, PT, C).transpose(1, 0, 2)
         .reshape(P, C) for i in range(N_CORES)], axis=0)
    if _trace:
        return outp, res
    return outp


# revision 40
# speedup vs baseline: 1.0256x; 1.0256x over previous
"""DocRED relation-extraction head on 8 Trainium2 NeuronCores (~39us).

Data-parallel over the batch axis: core b owns batch b's hidden_states slab
and its entity/pair indices; the classifier weights are replicated.

Instead of materializing rel = concat(subj, obj) [P, 2H] and computing
[P,2H] @ [2H,H] @ [H,97] per pair (32x redundant: only 32 distinct entities
feed 1024 pairs), project the entities first and gather at the end:

    repT    = mention-sum of 128 hidden rows, via indirect-DMA gather +
              8 matmuls against a block-ones matrix (fuses sum + transpose)
    proj    = rep @ [W1 | W2]   W streamed as the MOVING operand in N=512
              chunks (16 x 256KB fp16 slabs, the dominant DMA stream, at the
              HBM-pair roofline); 4 PSUM bank accumulators
    projT   = proj flipped via 2 batched DVE 32x32 stream-transposes per
              half + 4 partition-restack SBUF->SBUF DMAs
    eL1'    = [projT1 | dense_b].T @ out_w  [33, 98]  (row 32 = const row)
    eL2     = projT2.T @ out_w              [32, 98]
    logits[p] = eL1'[head[p]] + const + eL2[tail[p]], via ONE K=65-stacked
              one-hot matmul per 128-pair tile: rows 0-31 head one-hot,
              32-63 tail one-hot, 64 all-ones (adds const+out_b once).

Precision: fp32 matmul on trn2 lowers to 2 ISA passes (hi/lo), so matmul
inputs use float32r (single-pass, ~1.3e-4 rel per matmul), and dense_w /
hidden_states travel as fp16 (their magnitudes are well inside fp16 range;
halves the dominant DMA bytes). End-to-end vs the fp32 reference: ~4e-4
scale-relative error. Set MM_DT = f32 / W_DT = f32 for exact fp32 (~77us).
"""

import numpy as np
from contextlib import ExitStack

import concourse.bass as bass
import concourse.bacc as bacc
import concourse.tile as tile
import concourse.mybir as mybir
from concourse.bass_utils import run_bass_kernel_spmd

B, L, H, E, M, P, C = 8, 2048, 1024, 32, 4, 1024, 97
N_CORES = 8
HC = H // 128   # h-dim chunks (contraction of dense)
JC = H // 128   # j-dim chunks (output of dense / contraction of out proj)
PT = P // 128   # pair tiles
SLOT = E + 1    # projT slot width: 32 cols projT + 1 col dense_b chunk

f32 = mybir.dt.float32
f32r = mybir.dt.float32r
i32 = mybir.dt.int32

MM_DT = f32r    # dtype of matmul input tiles (f32r: 1-pass PE, ~1e-4 rel err)
W_DT = mybir.dt.float16  # dense_w + repT dtype: halves the dominant DMA
                         # stream (4MB vs 8MB); W values are ~N(0, 1/2048) so
                         # fp16's 10-bit mantissa costs ~5e-4 rel

CP = C + 1                # class dim padded to 98: f32r needs an even
                          # moving dim; the pad column is zero end to end

# constant-blob column layout
ONES0 = 1                 # [128, 32] mention-sum block-ones
DB0 = ONES0 + E           # [128, 8] dense_b chunks
OW0 = DB0 + HC            # [128, 8*98] out_w chunks (zero pad col each)
IOTA0 = OW0 + JC * CP     # [32, 1] iota column
OB0 = IOTA0 + 1           # [1, 98] out_b on row 0 (zero padded)
BLOBW = OB0 + CP

_CACHE = {}


def _build():
    nc = bacc.Bacc("TRN2", target_bir_lowering=False, debug=False)

    hs = nc.dram_tensor("hs", [L, H], W_DT, kind="ExternalInput").ap()
    pos = nc.dram_tensor("pos", [E * M, 1], i32, kind="ExternalInput").ap()
    onesb = nc.dram_tensor("onesb", [E * M, E], W_DT, kind="ExternalInput").ap()
    blob = nc.dram_tensor("blob", [128, BLOBW], MM_DT, kind="ExternalInput").ap()
    headrep = nc.dram_tensor("headrep", [E, P], MM_DT, kind="ExternalInput").ap()
    tailrep = nc.dram_tensor("tailrep", [E, P], MM_DT, kind="ExternalInput").ap()
    dw = nc.dram_tensor("dw", [2 * H, H], W_DT, kind="ExternalInput").ap()
    owt = nc.dram_tensor("owt", [128, JC * CP], MM_DT, kind="ExternalInput").ap()
    # output laid out [128, PT*C]: pair-tile t in columns t*C..(t+1)*C; one
    # contiguous DMA out, host reshapes to [P, C]
    out = nc.dram_tensor("out", [128, PT * C], f32, kind="ExternalOutput").ap()

    with tile.TileContext(nc) as tc, ExitStack() as ctx:
        sb = ctx.enter_context(tc.tile_pool(name="sb", bufs=1))
        wpool = ctx.enter_context(tc.tile_pool(name="w", bufs=12))
        opool = ctx.enter_context(tc.tile_pool(name="o", bufs=2))
        # One accumulation group per PSUM bank at a time (start=True clears
        # has_written for the whole bank) -> single 8-slot pool, bank per slot.
        pspool = ctx.enter_context(tc.tile_pool(name="ps", bufs=8, space="PSUM"))

        # ---- latency-critical inputs: pos + ones lead the SYNC ring so the
        # gather + stage A can run before the W stream floods the SDMA
        # engines; everything else rides the scalar ring.
        sb_pos = sb.tile([E * M, 1], i32)
        nc.sync.dma_start(sb_pos[:], pos[:])
        sb_ones = sb.tile([E * M, E], W_DT)
        nc.sync.dma_start(sb_ones[:], onesb[:])
        sb_blob = sb.tile([128, BLOBW], MM_DT)
        nc.scalar.dma_start(sb_blob[:], blob[:])
        sb_hr = sb.tile([E, P], MM_DT)
        nc.scalar.dma_start(sb_hr[:], headrep[:])
        sb_tr = sb.tile([E, P], MM_DT)
        nc.scalar.dma_start(sb_tr[:], tailrep[:])

        # ---- gather the 128 mention rows of hidden_states
        sb_g = sb.tile([E * M, H], W_DT)
        nc.gpsimd.indirect_dma_start(
            out=sb_g[:],
            out_offset=None,
            in_=hs[:],
            in_offset=bass.IndirectOffsetOnAxis(ap=sb_pos[:, :1], axis=0),
        )

        # ---- PE warm-up: the HAM clock gate holds an idle PE at 1.2 GHz and
        # needs ~3.4us of sustained activity to release to 2.4 GHz. Burn
        # discarded matmuls on a memset tile (no DMA dependency -> starts as
        # soon as the PE preamble ends) so stage B runs at full clock. f32
        # dummies lower to 2 ISA passes each: fewer instructions per us.
        wdum = sb.tile([128, E], f32)
        nc.vector.memset(wdum[:], 0.0)
        ps_warm = pspool.tile([E, E], f32, tag="ps")
        for i in range(90):
            nc.tensor.matmul(
                out=ps_warm[:],
                lhsT=wdum[:],
                rhs=wdum[:],
                start=True,
                stop=True,
            )

        # ---- stage A: entity_repT[h, e] = sum_m gathered[4e+m, h]
        # (mention-sum and transpose fused into 8 matmuls vs block-ones)
        sb_repT = sb.tile([128, HC * E], W_DT)
        for hc in range(HC):
            pa = pspool.tile([128, E], f32, tag="ps", name=f"pa{hc}")
            nc.tensor.matmul(
                out=pa[:],
                lhsT=sb_g[:, hc * 128:(hc + 1) * 128],
                rhs=sb_ones[:],
                start=True,
                stop=True,
            )
            nc.vector.tensor_copy(out=sb_repT[:, hc * E:(hc + 1) * E], in_=pa[:])

        # ---- stage B: proj = rep @ [W1 | W2] with W as the moving operand.
        sb_projT = sb.tile([128, 2 * JC * SLOT], MM_DT)
        ps_eL1 = pspool.tile([SLOT, CP], f32, tag="ps")
        ps_eL2 = pspool.tile([E, CP], f32, tag="ps")
        # dense_b chunks ride along as lhsT column 32 of the half-0 slots
        for jc in range(JC):
            nc.vector.tensor_copy(
                out=sb_projT[:, jc * SLOT + E:jc * SLOT + E + 1],
                in_=sb_blob[:, DB0 + jc:DB0 + jc + 1],
            )
        # both halves of the W stream back-to-back on PE (no C in between --
        # C waits on the DVE transposes and would bubble the PE otherwise)
        ps_blk = [pspool.tile([E, 512], f32, tag="ps", name=f"ps_blk{i}")
                  for i in range(4)]
        for half in range(2):
            for hc in range(HC):
                wt = wpool.tile([128, H], W_DT, tag="wslab",
                                name=f"wt{half}_{hc}")
                nc.sync.dma_start(
                    wt[:], dw[(half * HC + hc) * 128:(half * HC + hc + 1) * 128, :])
                for q in range(2):
                    nc.tensor.matmul(
                        out=ps_blk[half * 2 + q][:],
                        lhsT=sb_repT[:, hc * E:(hc + 1) * E],
                        rhs=wt[:, q * 512:(q + 1) * 512],
                        start=(hc == 0),
                        stop=(hc == HC - 1),
                    )
        # out_w rides the sync ring BEHIND the W slabs: it is not needed
        # until stage C (~3us after the last slab) and must not steal HBM
        # bandwidth from the critical W stream.
        sb_ow = sb.tile([128, JC * CP], MM_DT)
        nc.sync.dma_start(sb_ow[:], owt[:])
        # per half: 2 batched 32x32-block stream-transposes (16 blocks each),
        # then 4 small SBUF->SBUF DMAs restack the blocks into partitions
        # (the DMA also retypes f32 -> f32r; same 4-byte lanes)
        for half in range(2):
            stT = sb.tile([E, H], f32, name=f"stT{half}")
            for q in range(2):
                nc.vector.transpose(
                    out=stT[:, q * 512:(q + 1) * 512],
                    in_=ps_blk[half * 2 + q][:])
            hs0 = half * JC * SLOT
            st4 = stT[:].bitcast(MM_DT).rearrange("p (j b c) -> p j b c", b=4, c=32)
            for bl in range(4):
                # half 0 runs mid-W-stream: keep its restacks off the sync
                # ring (FIFO behind the W slabs). At half 1 both rings idle.
                eng = nc.sync if (half == 1 and bl % 2 == 1) else nc.scalar
                eng.dma_start(
                    out=sb_projT[bl * 32:(bl + 1) * 32, hs0:hs0 + JC * SLOT]
                    .rearrange("p (j s) -> p j s", s=SLOT)[:, :, :E],
                    in_=st4[:, :, bl, :],
                )
            eL, w_m = (ps_eL1, SLOT) if half == 0 else (ps_eL2, E)
            for jc in range(JC):
                slot = (half * JC + jc) * SLOT
                nc.tensor.matmul(
                    out=eL[:],
                    lhsT=sb_projT[:, slot:slot + w_m],
                    rhs=sb_ow[:, jc * CP:(jc + 1) * CP],
                    start=(jc == 0),
                    stop=(jc == JC - 1),
                )

        # ---- eL stack [65, 97] (partition offsets must be 32-aligned):
        # rows 0-31 = eL1, rows 32-63 = eL2, row 64 = dense_b @ out_w + out_b.
        sb_eL = sb.tile([2 * E + 1, CP], MM_DT)
        nc.vector.tensor_copy(out=sb_eL[:E, :], in_=ps_eL1[:E, :])
        nc.vector.tensor_copy(out=sb_eL[E:2 * E, :], in_=ps_eL2[:])
        nc.vector.tensor_add(
            out=sb_eL[2 * E:2 * E + 1, :], in0=ps_eL1[E:E + 1, :],
            in1=sb_blob[:1, OB0:OB0 + CP])

        # ---- stage D: stacked one-hot pair gather.
        # K rows 0-31: head one-hot; rows 32-63: tail; row 64: ones (const).
        sb_oh = sb.tile([2 * E + 1, P], MM_DT)
        nc.vector.tensor_tensor(
            out=sb_oh[:E, :],
            in0=sb_blob[:E, IOTA0:IOTA0 + 1].to_broadcast([E, P]),
            in1=sb_hr[:],
            op=mybir.AluOpType.is_equal,
        )
        nc.vector.tensor_tensor(
            out=sb_oh[E:2 * E, :],
            in0=sb_blob[:E, IOTA0:IOTA0 + 1].to_broadcast([E, P]),
            in1=sb_tr[:],
            op=mybir.AluOpType.is_equal,
        )
        # all-ones row via x==x (memset on an f32r tile fails the ISA check)
        nc.vector.tensor_tensor(
            out=sb_oh[2 * E:2 * E + 1, :],
            in0=sb_blob[:1, IOTA0:IOTA0 + 1].to_broadcast([1, P]),
            in1=sb_blob[:1, IOTA0:IOTA0 + 1].to_broadcast([1, P]),
            op=mybir.AluOpType.is_equal,
        )
        sb_out = opool.tile([128, PT * C], f32, bufs=1)
        for pt in range(PT):
            pl = pspool.tile([128, CP], f32, tag="ps", name=f"pl{pt}")
            nc.tensor.matmul(
                out=pl[:],
                lhsT=sb_oh[:, pt * 128:(pt + 1) * 128],
                rhs=sb_eL[:],
                start=True,
                stop=True,
            )
            nc.vector.tensor_copy(
                out=sb_out[:, pt * C:(pt + 1) * C], in_=pl[:, :C])
        # single contiguous store; host reshapes [128, 8*97] -> [1024, 97]
        nc.scalar.dma_start(out[:], sb_out[:])

    nc.compile()
    return nc


def get_compiled():
    if "nc" not in _CACHE:
        _CACHE["nc"] = _build()
    return _CACHE["nc"]


def make_in_maps(hidden_states, dense_w, dense_b, out_w, out_b,
                 entity_position_ids, head_tail_idxs):
    # inputs may arrive as jax arrays; normalize to host numpy first
    hidden_states = np.asarray(hidden_states)
    dense_w = np.asarray(dense_w)
    dense_b = np.asarray(dense_b)
    out_w = np.asarray(out_w)
    out_b = np.asarray(out_b)
    entity_position_ids = np.asarray(entity_position_ids)
    head_tail_idxs = np.asarray(head_tail_idxs)
    blob = np.zeros((128, BLOBW), np.float32)
    blob[:, DB0:DB0 + HC] = np.asarray(dense_b, np.float32).reshape(HC, 128).T
    owp = np.zeros((H, CP), np.float32)
    owp[:, :C] = np.asarray(out_w, np.float32)
    owt = np.ascontiguousarray(
        owp.reshape(JC, 128, CP).transpose(1, 0, 2).reshape(128, JC * CP))
    blob[:E, IOTA0] = np.arange(E, dtype=np.float32)
    blob[0, OB0:OB0 + C] = np.asarray(out_b, np.float32)  # col 97 stays 0
    dense_w = np.ascontiguousarray(dense_w, dtype=np.float16)
    in_maps = []
    for b in range(B):
        ht = head_tail_idxs[b].astype(np.float32)  # [P, 2]
        in_maps.append({
            "hs": np.ascontiguousarray(hidden_states[b], dtype=np.float16),
            "pos": np.ascontiguousarray(
                entity_position_ids[b].reshape(E * M, 1).astype(np.int32)),
            "onesb": np.repeat(np.eye(E, dtype=np.float16), M, axis=0),
            "blob": blob,
            "owt": owt,
            "headrep": np.ascontiguousarray(
                np.broadcast_to(ht[None, :, 0], (E, P))),
            "tailrep": np.ascontiguousarray(
                np.broadcast_to(ht[None, :, 1], (E, P))),
            "dw": dense_w,
        })
    return in_maps


def kernel(hidden_states, dense_w, dense_b, out_w, out_b,
           entity_position_ids, head_tail_idxs, _trace=False, _trace_kwargs=None):
    nc = get_compiled()
    in_maps = make_in_maps(hidden_states, dense_w, dense_b, out_w, out_b,
                           entity_position_ids, head_tail_idxs)
    res = run_bass_kernel_spmd(
        nc, in_maps, core_ids=list(range(N_CORES)),
        trace=_trace, **(_trace_kwargs or {}),
    )
    outp = np.concatenate(
        [res.results[i]["out"].reshape(128, PT, C).transpose(1, 0, 2)
         .reshape(P, C) for i in range(N_CORES)], axis=0)
    if _trace:
        return outp, res
    return outp
